# revision 30
# baseline (speedup 1.0000x reference)
"""MCR2 variational loss on 8 Trainium2 NeuronCores.

Strategy (data-parallel over the sample axis n):
  - The heavy part of the loss is the per-class second-moment matrices
    M_j = Z^T diag(Pi_j) Z (plus the global gram Z^T Z), which reads all of
    Z/Pi once -> memory-bound. Everything downstream (logdet, log1p terms,
    Frobenius distance) is O(C*d^2) scalar work done on the host in fp64.
  - Fast path (Pi exactly one-hot): each sample contributes to exactly one
    class, so per-class partial grams over class-sorted rows give all M_j,
    and gram = sum_j M_j. Host distributes rows so every core gets an
    almost equal share of each class, pads each class segment to a 128-row
    multiple, and the device accumulates each class's Gram in PSUM.
  - Z ships as fp8 e4m3 (quarter of fp32 HBM traffic; measured effect on
    the final losses is ~1.5e-3 relative, an order of magnitude under the
    tolerance). The PE consumes row-chunks two at a time with the fp8
    DoubleRow perf mode (2 rows/cycle); odd segment tails use a single
    plain fp8 matmul.
  - DMA: each per-core stream is cut into ramped blocks (small first so
    the PE starts early) interleaved across the two HWDGE rings (SP + ACT,
    ~150 GB/s each) with bytes balanced so both rings run to the end.
    Blocks are pre-tiled in DRAM so every SBUF partition's data is one
    contiguous descriptor.
  - Output: per-class partial M drains from PSUM as bf16 (cast on the DVE
    copy) per PSUM bank group; mid-stream stores ride SWDGE so they never
    steal load-ring bandwidth, the final store uses the ACT ring (loads
    long done). Host all-reduces the 8 partials in fp64.
  - Fallback (general dense Pi): host BLAS contraction.
"""

import numpy as np

EPS = 0.5
MU = 1.0
C = 10
N_TOTAL = 131072
D = 128
N_CORES = 8
CHUNK = 128  # rows per PE k-tile (contraction dim)

_compiled_cache = {}

# Every input block is split across the two HWDGE rings by partition
# halves (SP/sync gets SBUF partitions 0-63, ACT/scalar 64-127); SWDGE is
# ~3x slower for loads so it only carries the small mid-stream output
# stores. A ring processes one descriptor (= one SBUF partition row)
# every ~14 ns OR descriptor_bytes/145 ns, whichever is larger — so with
# 16-chunk blocks (2KB descriptors) each ring spends ~0.9us per 64-row
# half, the rings stay byte-balanced by construction, and block flight
# latency is half that of whole-block-per-ring scheduling. ~8 blocks also
# keeps DMAs per engine inside the semaphore pool (reuse serializes
# issue).
BLOCK_CHUNKS = 16


def _plan(seg_chunks):
    """PE op list + DMA block schedule for a class-sorted chunk stream.

    Returns (blocks, queue_of_block): blocks is a list of
    (ops, n_chunks) where each op is a dict with cls / w (1 or 2 chunks) /
    q (chunk offset inside the block) / start / stop. Pairs (w=2) never
    straddle a block boundary so each DoubleRow matmul reads one tile.
    """
    ops = []
    for j, k in enumerate(seg_chunks):
        n2, n1 = divmod(k, 2)
        widths = [2] * n2 + ([1] if n1 else [])
        for i, w in enumerate(widths):
            ops.append(
                {"cls": j, "w": w, "start": i == 0, "stop": i == len(widths) - 1}
            )
    total = sum(seg_chunks)

    # uniform blocks; the final two absorb the remainder equally so the
    # rings stay byte-balanced
    n_blocks = max(2, total // BLOCK_CHUNKS)
    targets = [BLOCK_CHUNKS] * (n_blocks - 2)

    # pack ops to the ramp/steady targets; whatever remains is split into
    # two equal final blocks so the rings stay byte-balanced
    blocks = []
    ti = 0
    cur, cur_chunks = [], 0
    oi = 0
    while oi < len(ops) and ti < len(targets):
        op = dict(ops[oi])
        if cur and cur_chunks + op["w"] > targets[ti]:
            blocks.append((cur, cur_chunks))
            ti += 1
            cur, cur_chunks = [], 0
            continue
        op["q"] = cur_chunks
        cur.append(op)
        cur_chunks += op["w"]
        oi += 1
    rest_ops = ops[oi:]
    rest_chunks = sum(op["w"] for op in rest_ops)
    if cur and not rest_ops:
        blocks.append((cur, cur_chunks))
    elif rest_ops:
        if cur:
            blocks.append((cur, cur_chunks))
        half = rest_chunks // 2
        cur, cur_chunks = [], 0
        split_done = False
        for op in rest_ops:
            if not split_done and cur_chunks >= half:
                blocks.append((cur, cur_chunks))
                cur, cur_chunks = [], 0
                split_done = True
            op = dict(op)
            op["q"] = cur_chunks
            cur.append(op)
            cur_chunks += op["w"]
        blocks.append((cur, cur_chunks))

    return blocks


def _build_bass_program(seg_chunks):
    """SPMD bass program computing per-class partial grams in fp8.

    Device input "z": class-sorted, zero-padded, PRE-TILED fp8 Z — for each
    DMA block of kb chunks a contiguous [128, kb*128] slab (each SBUF
    partition's data contiguous in DRAM). Output "m_out": [128, C*128]
    bf16 partial M (d on partitions, (j,e) on free).
    """
    import concourse.bacc as bacc
    import concourse.tile as tile
    from concourse import mybir
    from contextlib import ExitStack

    blocks = _plan(seg_chunks)
    total_chunks = sum(kb for _, kb in blocks)

    # Each PSUM bank is its OWN tile: classes 0-3 bank0, 4-7 bank1,
    # 8 bank2, 9 bank3. Draining a finished bank must not create a
    # write-after-read hazard on the banks the PE is still accumulating
    # into — with one big acc tile the Tile framework serializes every
    # subsequent matmul behind the drain copy (~1.4us stall per drain).
    # bank index, column offset within bank (in classes)
    psum_bank = {j: (0, j) for j in range(4)}
    psum_bank.update({j: (1, j - 4) for j in range(4, 8)})
    psum_bank[8] = (2, 0)
    psum_bank[9] = (3, 0)
    # store units: class range [a, b) stored when class b-1's cast is done.
    # Classes 8+9 ship as ONE ring-split store: flight time is descriptor-
    # count bound, so two half-stores per ring would cost double.
    stores = {3: (0, 4), 7: (4, 8), 9: (8, 10)}

    nc = bacc.Bacc("TRN2", target_bir_lowering=False, debug=False, num_devices=N_CORES)
    z = nc.dram_tensor(
        "z", [total_chunks * CHUNK, D], mybir.dt.float8e4, kind="ExternalInput"
    ).ap()
    out = nc.dram_tensor(
        "m_out", [D, C * D], mybir.dt.bfloat16, kind="ExternalOutput"
    ).ap()

    with tile.TileContext(nc) as tc:
        with ExitStack() as ctx:
            psum = ctx.enter_context(tc.tile_pool(name="psum", bufs=1, space="PSUM"))
            opool = ctx.enter_context(tc.tile_pool(name="o", bufs=1))
            banks = [
                psum.tile([128, 4 * D], mybir.dt.float32, name=f"bank{i}")
                for i in range(4)
            ]
            sb_out = opool.tile([128, C * D], mybir.dt.bfloat16)
            row0 = 0
            for b, (ops, kb) in enumerate(blocks):
                pool = ctx.enter_context(tc.tile_pool(name=f"z{b}", bufs=1))
                tl = pool.tile([128, kb * D], mybir.dt.float8e4)
                src = z[row0 : row0 + CHUNK * kb, :].rearrange(
                    "(p k) d -> p (k d)", p=128
                )
                nc.sync.dma_start(tl[0:64, :], src[0:64, :])
                nc.scalar.dma_start(tl[64:128, :], src[64:128, :])
                row0 += CHUNK * kb
                for op in ops:
                    j, q = op["cls"], op["q"]
                    bk, col = psum_bank[j]
                    dst = banks[bk][:, col * D : (col + 1) * D]
                    if op["w"] == 2:
                        opnd = tl[:, q * D : (q + 2) * D].rearrange(
                            "p (two d) -> p two d", two=2
                        )
                        nc.tensor.matmul(
                            dst,
                            opnd,
                            opnd,
                            start=op["start"],
                            stop=op["stop"],
                            perf_mode=mybir.MatmulPerfMode.DoubleRow,
                            skip_group_check=True,
                        )
                    else:
                        opnd = tl[:, q * D : (q + 1) * D]
                        nc.tensor.matmul(
                            dst,
                            opnd,
                            opnd,
                            start=op["start"],
                            stop=op["stop"],
                            skip_group_check=True,
                        )
                    # cast each class out of PSUM the moment it finishes so
                    # only the last class's short cast sits on the critical
                    # path; mid-stream stores ride SWDGE so they never
                    # steal load-ring bandwidth
                    if op["stop"]:
                        bk, col = psum_bank[j]
                        csl = slice(j * D, (j + 1) * D)
                        nc.vector.tensor_copy(
                            sb_out[:, csl], banks[bk][:, col * D : (col + 1) * D]
                        )
                        if j in stores:
                            a, bcls = stores[j]
                            sl = slice(a * D, bcls * D)
                            if j < C - 1:
                                nc.gpsimd.dma_start(out[:, sl], sb_out[:, sl])
                            else:
                                # the final store is the one on the critical
                                # path and its flight time is descriptor-
                                # count bound, so split its partition rows
                                # across both HW rings (input long done)
                                nc.sync.dma_start(
                                    out[0:64, sl], sb_out[0:64, sl]
                                )
                                nc.scalar.dma_start(
                                    out[64:128, sl], sb_out[64:128, sl]
                                )
    nc.compile()
    return nc


def _is_one_hot(Pi):
    if not (Pi.sum(axis=1) == 1.0).all():
        return False
    if not (Pi.max(axis=1) == 1.0).all():
        return False
    return np.count_nonzero(Pi) == Pi.shape[0]


def _fast_path_M(Z, Pi):
    """Per-class second moments via the device. Returns M [C, D, D] fp64."""
    from concourse.bass_utils import run_bass_kernel_spmd
    from concourse import mybir

    fp8 = mybir.dt.np(mybir.dt.float8e4)
    labels = np.argmax(Pi, axis=1)

    # balance every class across cores: class j's rows are dealt out in
    # near-equal contiguous slices, so per-class per-core counts differ by
    # at most 1 and padding is minimal
    order = np.argsort(labels, kind="stable")
    cls_counts = np.bincount(labels, minlength=C)
    cls_offs = np.concatenate([[0], np.cumsum(cls_counts)])

    counts = np.zeros((N_CORES, C), dtype=np.int64)
    for j in range(C):
        base, rem = divmod(int(cls_counts[j]), N_CORES)
        for c in range(N_CORES):
            counts[c, j] = base + (1 if c < rem else 0)

    seg_chunks = [max(1, int(np.ceil(counts[:, j].max() / CHUNK))) for j in range(C)]

    key = tuple(seg_chunks)
    if key not in _compiled_cache:
        _compiled_cache[key] = _build_bass_program(seg_chunks)
    nc = _compiled_cache[key]

    blocks = _plan(seg_chunks)
    block_sizes = [kb for _, kb in blocks]
    total_chunks = sum(block_sizes)
    offs = np.concatenate([[0], np.cumsum(seg_chunks)]) * CHUNK

    Zq = np.ascontiguousarray(Z, dtype=np.float32).astype(fp8)
    in_maps = []
    for c in range(N_CORES):
        zbuf = np.zeros((total_chunks * CHUNK, D), dtype=fp8)
        for j in range(C):
            lo = cls_offs[j] + counts[:c, j].sum()
            nj = counts[c, j]
            zbuf[offs[j] : offs[j] + nj] = Zq[order[lo : lo + nj]]
        # pre-tile each DMA block: [kb, 128, D] -> [128, kb*D]
        parts = []
        start = 0
        for kb in block_sizes:
            blk = zbuf[start * CHUNK : (start + kb) * CHUNK]
            parts.append(
                np.ascontiguousarray(
                    blk.reshape(kb, CHUNK, D).transpose(1, 0, 2)
                ).reshape(-1)
            )
            start += kb
        zdev = np.concatenate(parts).reshape(total_chunks * CHUNK, D)
        in_maps.append({"z": zdev})

    res = run_bass_kernel_spmd(nc, in_maps, list(range(N_CORES)))
    M = np.zeros((C, D, D), dtype=np.float64)
    for c in range(N_CORES):
        o = res.results[c]["m_out"].astype(np.float64)  # [D, C*D]
        M += o.reshape(D, C, D).transpose(1, 0, 2)
    return M


def _dense_path_M(Z, Pi):
    """General dense Pi: host BLAS contraction. Returns (M, gram) fp64."""
    Zf = np.ascontiguousarray(Z, dtype=np.float32)
    A = (Pi[:, :, None].astype(np.float32) * Zf[:, None, :]).reshape(Zf.shape[0], -1)
    M = (A.T @ Zf).reshape(C, D, D).astype(np.float64)
    gram = (Zf.T @ Zf).astype(np.float64)
    return M, gram


def kernel(Z, Pi, Us):
    Z = np.asarray(Z, dtype=np.float32)
    Pi = np.asarray(Pi, dtype=np.float32)
    Us = np.asarray(Us, dtype=np.float32)
    n, d = Z.shape

    if n == N_TOTAL and d == D and Pi.shape == (n, C) and _is_one_hot(Pi):
        M = _fast_path_M(Z, Pi)
        gram = M.sum(axis=0)
    else:
        M, gram = _dense_path_M(Z, Pi)

    nf = float(n)
    df = float(d)

    A = np.eye(d, dtype=np.float64) + (df / (nf * EPS)) * gram
    sign, logabsdet = np.linalg.slogdet(A)
    loss_R = 0.5 * logabsdet

    trPi = Pi.astype(np.float64).sum(axis=0)
    col_norms_sq = (Us.astype(np.float64) ** 2).sum(axis=1)  # [C, d]
    with np.errstate(divide="ignore"):
        per_class = np.log1p((df / (trPi[:, None] * EPS)) * col_norms_sq).sum(axis=1)
    loss_Rc = ((trPi / (2.0 * nf)) * per_class).sum()

    Us64 = Us.astype(np.float64)
    UUt = np.einsum("jdk,jek->jde", Us64, Us64)
    loss_reg = 0.5 * MU * ((M - UUt) ** 2).sum()

    loss_obj = loss_R - loss_Rc - loss_reg
    return (
        np.float32(-loss_obj),
        np.float32(loss_R),
        np.float32(loss_Rc),
        np.float32(loss_reg),
    )


# revision 31
# speedup vs baseline: 1.1180x; 1.1180x over previous
"""MCR2 variational loss on 8 Trainium2 NeuronCores.

Strategy (data-parallel over the sample axis n):
  - The heavy part of the loss is the per-class second-moment matrices
    M_j = Z^T diag(Pi_j) Z (plus the global gram Z^T Z), which reads all of
    Z/Pi once -> memory-bound. Everything downstream (logdet, log1p terms,
    Frobenius distance) is O(C*d^2) scalar work done on the host in fp64.
  - Fast path (Pi exactly one-hot): each sample contributes to exactly one
    class, so per-class partial grams over class-sorted rows give all M_j,
    and gram = sum_j M_j. Host distributes rows so every core gets an
    almost equal share of each class, pads each class segment to a 128-row
    multiple, and the device accumulates each class's Gram in PSUM.
  - Z ships as fp8 e4m3 (quarter of fp32 HBM traffic; measured effect on
    the final losses is ~1.5e-3 relative, an order of magnitude under the
    tolerance). The PE consumes row-chunks two at a time with the fp8
    DoubleRow perf mode (2 rows/cycle); odd segment tails use a single
    plain fp8 matmul.
  - DMA: each per-core stream is cut into ramped blocks (small first so
    the PE starts early) interleaved across the two HWDGE rings (SP + ACT,
    ~150 GB/s each) with bytes balanced so both rings run to the end.
    Blocks are pre-tiled in DRAM so every SBUF partition's data is one
    contiguous descriptor.
  - Output: per-class partial M drains from PSUM as bf16 (cast on the DVE
    copy) per PSUM bank group; mid-stream stores ride SWDGE so they never
    steal load-ring bandwidth, the final store uses the ACT ring (loads
    long done). Host all-reduces the 8 partials in fp64.
  - Fallback (general dense Pi): host BLAS contraction.
"""

import numpy as np

EPS = 0.5
MU = 1.0
C = 10
N_TOTAL = 131072
D = 128
N_CORES = 8
CHUNK = 128  # rows per PE k-tile (contraction dim)

_compiled_cache = {}

# Every input block is split across the two HWDGE rings by partition
# halves (SP/sync gets SBUF partitions 0-63, ACT/scalar 64-127); SWDGE is
# ~3x slower for loads so it only carries the small mid-stream output
# stores. A ring processes one descriptor (= one SBUF partition row)
# every ~14 ns OR descriptor_bytes/145 ns, whichever is larger — so with
# 16-chunk blocks (2KB descriptors) each ring spends ~0.9us per 64-row
# half, the rings stay byte-balanced by construction, and block flight
# latency is half that of whole-block-per-ring scheduling. ~8 blocks also
# keeps DMAs per engine inside the semaphore pool (reuse serializes
# issue).
BLOCK_CHUNKS = 16


def _plan(seg_chunks):
    """PE op list + DMA block schedule for a class-sorted chunk stream.

    Returns (blocks, queue_of_block): blocks is a list of
    (ops, n_chunks) where each op is a dict with cls / w (1 or 2 chunks) /
    q (chunk offset inside the block) / start / stop. Pairs (w=2) never
    straddle a block boundary so each DoubleRow matmul reads one tile.
    """
    ops = []
    for j, k in enumerate(seg_chunks):
        n2, n1 = divmod(k, 2)
        widths = [2] * n2 + ([1] if n1 else [])
        for i, w in enumerate(widths):
            ops.append(
                {"cls": j, "w": w, "start": i == 0, "stop": i == len(widths) - 1}
            )
    total = sum(seg_chunks)

    # uniform blocks; the final two absorb the remainder equally so the
    # rings stay byte-balanced
    n_blocks = max(2, total // BLOCK_CHUNKS)
    targets = [BLOCK_CHUNKS] * (n_blocks - 2)

    # pack ops to the ramp/steady targets; whatever remains is split into
    # two equal final blocks so the rings stay byte-balanced
    blocks = []
    ti = 0
    cur, cur_chunks = [], 0
    oi = 0
    while oi < len(ops) and ti < len(targets):
        op = dict(ops[oi])
        if cur and cur_chunks + op["w"] > targets[ti]:
            blocks.append((cur, cur_chunks))
            ti += 1
            cur, cur_chunks = [], 0
            continue
        op["q"] = cur_chunks
        cur.append(op)
        cur_chunks += op["w"]
        oi += 1
    rest_ops = ops[oi:]
    rest_chunks = sum(op["w"] for op in rest_ops)
    if cur and not rest_ops:
        blocks.append((cur, cur_chunks))
    elif rest_ops:
        if cur:
            blocks.append((cur, cur_chunks))
        half = rest_chunks // 2
        cur, cur_chunks = [], 0
        split_done = False
        for op in rest_ops:
            if not split_done and cur_chunks >= half:
                blocks.append((cur, cur_chunks))
                cur, cur_chunks = [], 0
                split_done = True
            op = dict(op)
            op["q"] = cur_chunks
            cur.append(op)
            cur_chunks += op["w"]
        blocks.append((cur, cur_chunks))

    return blocks


def _build_bass_program(seg_chunks):
    """SPMD bass program computing per-class partial grams in fp8.

    Device input "z": class-sorted, zero-padded, PRE-TILED fp8 Z — for each
    DMA block of kb chunks a contiguous [128, kb*128] slab (each SBUF
    partition's data contiguous in DRAM). Output "m_out": [128, C*128]
    bf16 partial M (d on partitions, (j,e) on free).
    """
    import concourse.bacc as bacc
    import concourse.tile as tile
    from concourse import mybir
    from contextlib import ExitStack

    blocks = _plan(seg_chunks)
    total_chunks = sum(kb for _, kb in blocks)

    # Each PSUM bank is its OWN tile: classes 0-3 bank0, 4-7 bank1,
    # 8 bank2, 9 bank3. Draining a finished bank must not create a
    # write-after-read hazard on the banks the PE is still accumulating
    # into — with one big acc tile the Tile framework serializes every
    # subsequent matmul behind the drain copy (~1.4us stall per drain).
    # bank index, column offset within bank (in classes)
    psum_bank = {j: (0, j) for j in range(4)}
    psum_bank.update({j: (1, j - 4) for j in range(4, 8)})
    psum_bank[8] = (2, 0)
    psum_bank[9] = (3, 0)
    # store units: class range [a, b) stored when class b-1's cast is done.
    # Classes 8+9 ship as ONE ring-split store: flight time is descriptor-
    # count bound, so two half-stores per ring would cost double.
    stores = {3: (0, 4), 7: (4, 8), 9: (8, 10)}

    nc = bacc.Bacc("TRN2", target_bir_lowering=False, debug=False, num_devices=N_CORES)
    z = nc.dram_tensor(
        "z", [total_chunks * CHUNK, D], mybir.dt.float8e4, kind="ExternalInput"
    ).ap()
    out = nc.dram_tensor(
        "m_out", [D, C * D], mybir.dt.bfloat16, kind="ExternalOutput"
    ).ap()

    with tile.TileContext(nc) as tc:
        with ExitStack() as ctx:
            psum = ctx.enter_context(tc.tile_pool(name="psum", bufs=1, space="PSUM"))
            opool = ctx.enter_context(tc.tile_pool(name="o", bufs=1))
            banks = [
                psum.tile([128, 4 * D], mybir.dt.float32, name=f"bank{i}")
                for i in range(4)
            ]
            sb_out = opool.tile([128, C * D], mybir.dt.bfloat16)
            row0 = 0
            for b, (ops, kb) in enumerate(blocks):
                pool = ctx.enter_context(tc.tile_pool(name=f"z{b}", bufs=1))
                tl = pool.tile([128, kb * D], mybir.dt.float8e4)
                src = z[row0 : row0 + CHUNK * kb, :].rearrange(
                    "(p k) d -> p (k d)", p=128
                )
                nc.sync.dma_start(tl[0:64, :], src[0:64, :])
                nc.scalar.dma_start(tl[64:128, :], src[64:128, :])
                row0 += CHUNK * kb
                for op in ops:
                    j, q = op["cls"], op["q"]
                    bk, col = psum_bank[j]
                    dst = banks[bk][:, col * D : (col + 1) * D]
                    if op["w"] == 2:
                        opnd = tl[:, q * D : (q + 2) * D].rearrange(
                            "p (two d) -> p two d", two=2
                        )
                        nc.tensor.matmul(
                            dst,
                            opnd,
                            opnd,
                            start=op["start"],
                            stop=op["stop"],
                            perf_mode=mybir.MatmulPerfMode.DoubleRow,
                            skip_group_check=True,
                        )
                    else:
                        opnd = tl[:, q * D : (q + 1) * D]
                        nc.tensor.matmul(
                            dst,
                            opnd,
                            opnd,
                            start=op["start"],
                            stop=op["stop"],
                            skip_group_check=True,
                        )
                    # cast a PSUM bank out only when the whole bank is done
                    # — casting a single class early would create a write-
                    # after-read hazard with the PE still accumulating the
                    # bank's other classes; mid-stream stores ride SWDGE so
                    # they never steal load-ring bandwidth
                    if op["stop"]:
                        bk, col = psum_bank[j]
                        if j in (3, 7, 8, 9):
                            a0 = {3: 0, 7: 4, 8: 8, 9: 9}[j]
                            bk, col = psum_bank[a0]
                            csl = slice(a0 * D, (j + 1) * D)
                            nc.vector.tensor_copy(
                                sb_out[:, csl],
                                banks[bk][:, col * D : (col + (j + 1 - a0)) * D],
                            )
                        if j in stores:
                            a, bcls = stores[j]
                            sl = slice(a * D, bcls * D)
                            if j < C - 1:
                                nc.gpsimd.dma_start(out[:, sl], sb_out[:, sl])
                            else:
                                # the final store is the one on the critical
                                # path and its flight time is descriptor-
                                # count bound, so split its partition rows
                                # across both HW rings (input long done)
                                nc.sync.dma_start(
                                    out[0:64, sl], sb_out[0:64, sl]
                                )
                                nc.scalar.dma_start(
                                    out[64:128, sl], sb_out[64:128, sl]
                                )
    nc.compile()
    return nc


def _is_one_hot(Pi):
    if not (Pi.sum(axis=1) == 1.0).all():
        return False
    if not (Pi.max(axis=1) == 1.0).all():
        return False
    return np.count_nonzero(Pi) == Pi.shape[0]


def _fast_path_M(Z, Pi):
    """Per-class second moments via the device. Returns M [C, D, D] fp64."""
    from concourse.bass_utils import run_bass_kernel_spmd
    from concourse import mybir

    fp8 = mybir.dt.np(mybir.dt.float8e4)
    labels = np.argmax(Pi, axis=1)

    # balance every class across cores: class j's rows are dealt out in
    # near-equal contiguous slices, so per-class per-core counts differ by
    # at most 1 and padding is minimal
    order = np.argsort(labels, kind="stable")
    cls_counts = np.bincount(labels, minlength=C)
    cls_offs = np.concatenate([[0], np.cumsum(cls_counts)])

    counts = np.zeros((N_CORES, C), dtype=np.int64)
    for j in range(C):
        base, rem = divmod(int(cls_counts[j]), N_CORES)
        for c in range(N_CORES):
            counts[c, j] = base + (1 if c < rem else 0)

    seg_chunks = [max(1, int(np.ceil(counts[:, j].max() / CHUNK))) for j in range(C)]

    key = tuple(seg_chunks)
    if key not in _compiled_cache:
        _compiled_cache[key] = _build_bass_program(seg_chunks)
    nc = _compiled_cache[key]

    blocks = _plan(seg_chunks)
    block_sizes = [kb for _, kb in blocks]
    total_chunks = sum(block_sizes)
    offs = np.concatenate([[0], np.cumsum(seg_chunks)]) * CHUNK

    Zq = np.ascontiguousarray(Z, dtype=np.float32).astype(fp8)
    in_maps = []
    for c in range(N_CORES):
        zbuf = np.zeros((total_chunks * CHUNK, D), dtype=fp8)
        for j in range(C):
            lo = cls_offs[j] + counts[:c, j].sum()
            nj = counts[c, j]
            zbuf[offs[j] : offs[j] + nj] = Zq[order[lo : lo + nj]]
        # pre-tile each DMA block: [kb, 128, D] -> [128, kb*D]
        parts = []
        start = 0
        for kb in block_sizes:
            blk = zbuf[start * CHUNK : (start + kb) * CHUNK]
            parts.append(
                np.ascontiguousarray(
                    blk.reshape(kb, CHUNK, D).transpose(1, 0, 2)
                ).reshape(-1)
            )
            start += kb
        zdev = np.concatenate(parts).reshape(total_chunks * CHUNK, D)
        in_maps.append({"z": zdev})

    res = run_bass_kernel_spmd(nc, in_maps, list(range(N_CORES)))
    M = np.zeros((C, D, D), dtype=np.float64)
    for c in range(N_CORES):
        o = res.results[c]["m_out"].astype(np.float64)  # [D, C*D]
        M += o.reshape(D, C, D).transpose(1, 0, 2)
    return M


def _dense_path_M(Z, Pi):
    """General dense Pi: host BLAS contraction. Returns (M, gram) fp64."""
    Zf = np.ascontiguousarray(Z, dtype=np.float32)
    A = (Pi[:, :, None].astype(np.float32) * Zf[:, None, :]).reshape(Zf.shape[0], -1)
    M = (A.T @ Zf).reshape(C, D, D).astype(np.float64)
    gram = (Zf.T @ Zf).astype(np.float64)
    return M, gram


def kernel(Z, Pi, Us):
    Z = np.asarray(Z, dtype=np.float32)
    Pi = np.asarray(Pi, dtype=np.float32)
    Us = np.asarray(Us, dtype=np.float32)
    n, d = Z.shape

    if n == N_TOTAL and d == D and Pi.shape == (n, C) and _is_one_hot(Pi):
        M = _fast_path_M(Z, Pi)
        gram = M.sum(axis=0)
    else:
        M, gram = _dense_path_M(Z, Pi)

    nf = float(n)
    df = float(d)

    A = np.eye(d, dtype=np.float64) + (df / (nf * EPS)) * gram
    sign, logabsdet = np.linalg.slogdet(A)
    loss_R = 0.5 * logabsdet

    trPi = Pi.astype(np.float64).sum(axis=0)
    col_norms_sq = (Us.astype(np.float64) ** 2).sum(axis=1)  # [C, d]
    with np.errstate(divide="ignore"):
        per_class = np.log1p((df / (trPi[:, None] * EPS)) * col_norms_sq).sum(axis=1)
    loss_Rc = ((trPi / (2.0 * nf)) * per_class).sum()

    Us64 = Us.astype(np.float64)
    UUt = np.einsum("jdk,jek->jde", Us64, Us64)
    loss_reg = 0.5 * MU * ((M - UUt) ** 2).sum()

    loss_obj = loss_R - loss_Rc - loss_reg
    return (
        np.float32(-loss_obj),
        np.float32(loss_R),
        np.float32(loss_Rc),
        np.float32(loss_reg),
    )


# revision 38
# speedup vs baseline: 1.1264x; 1.0075x over previous
"""MCR2 variational loss on 8 Trainium2 NeuronCores.

Strategy (data-parallel over the sample axis n):
  - The heavy part of the loss is the per-class second-moment matrices
    M_j = Z^T diag(Pi_j) Z (plus the global gram Z^T Z), which reads all of
    Z/Pi once -> memory-bound. Everything downstream (logdet, log1p terms,
    Frobenius distance) is O(C*d^2) scalar work done on the host in fp64.
  - Fast path (Pi exactly one-hot): each sample contributes to exactly one
    class, so per-class partial grams over class-sorted rows give all M_j,
    and gram = sum_j M_j. Host distributes rows so every core gets an
    almost equal share of each class, pads each class segment to a 128-row
    multiple, and the device accumulates each class's Gram in PSUM.
  - Z ships as fp8 e4m3 (quarter of fp32 HBM traffic; measured effect on
    the final losses is ~1.5e-3 relative, an order of magnitude under the
    tolerance). The PE consumes row-chunks two at a time with the fp8
    DoubleRow perf mode (2 rows/cycle); odd segment tails use a single
    plain fp8 matmul.
  - DMA: each per-core stream is cut into ramped blocks (small first so
    the PE starts early) interleaved across the two HWDGE rings (SP + ACT,
    ~150 GB/s each) with bytes balanced so both rings run to the end.
    Blocks are pre-tiled in DRAM so every SBUF partition's data is one
    contiguous descriptor.
  - Output: per-class partial M drains from PSUM as bf16 (cast on the DVE
    copy) per PSUM bank group; mid-stream stores ride SWDGE so they never
    steal load-ring bandwidth, the final store uses the ACT ring (loads
    long done). Host all-reduces the 8 partials in fp64.
  - Fallback (general dense Pi): host BLAS contraction.
"""

import numpy as np

EPS = 0.5
MU = 1.0
C = 10
N_TOTAL = 131072
D = 128
N_CORES = 8
CHUNK = 128  # rows per PE k-tile (contraction dim)

_compiled_cache = {}

# Every input block is split across the two HWDGE rings by partition
# halves (SP/sync gets SBUF partitions 0-63, ACT/scalar 64-127); SWDGE is
# ~3x slower for loads so it only carries the small mid-stream output
# stores. A ring processes one descriptor (= one SBUF partition row)
# every ~14 ns OR descriptor_bytes/145 ns, whichever is larger — so with
# 16-chunk blocks (2KB descriptors) each ring spends ~0.9us per 64-row
# half, the rings stay byte-balanced by construction, and block flight
# latency is half that of whole-block-per-ring scheduling. ~8 blocks also
# keeps DMAs per engine inside the semaphore pool (reuse serializes
# issue).
BLOCK_CHUNKS = 16


def _plan(seg_chunks):
    """PE op list + DMA block schedule for a class-sorted chunk stream.

    Returns (blocks, queue_of_block): blocks is a list of
    (ops, n_chunks) where each op is a dict with cls / w (1 or 2 chunks) /
    q (chunk offset inside the block) / start / stop. Pairs (w=2) never
    straddle a block boundary so each DoubleRow matmul reads one tile.
    """
    ops = []
    for j, k in enumerate(seg_chunks):
        n2, n1 = divmod(k, 2)
        widths = [2] * n2 + ([1] if n1 else [])
        for i, w in enumerate(widths):
            ops.append(
                {"cls": j, "w": w, "start": i == 0, "stop": i == len(widths) - 1}
            )
    total = sum(seg_chunks)

    # small first block (PE starts after its ~0.9us descriptor-floor
    # flight regardless of size), big middle blocks, small final pair (the
    # last block's flight directly precedes the final class's matmuls and
    # store; the splitter below turns the post-target remainder into two
    # equal blocks)
    first, mid = 12, 18
    n_mid = max(0, (total - first - 20) // mid)
    targets = [t for t in [first] + [mid] * n_mid if t > 0]

    # pack ops to the ramp/steady targets; whatever remains is split into
    # two equal final blocks so the rings stay byte-balanced
    blocks = []
    ti = 0
    cur, cur_chunks = [], 0
    oi = 0
    while oi < len(ops) and ti < len(targets):
        op = dict(ops[oi])
        if cur and cur_chunks + op["w"] > targets[ti]:
            blocks.append((cur, cur_chunks))
            ti += 1
            cur, cur_chunks = [], 0
            continue
        op["q"] = cur_chunks
        cur.append(op)
        cur_chunks += op["w"]
        oi += 1
    rest_ops = ops[oi:]
    rest_chunks = sum(op["w"] for op in rest_ops)
    if cur and not rest_ops:
        blocks.append((cur, cur_chunks))
    elif rest_ops:
        if cur:
            blocks.append((cur, cur_chunks))
        half = rest_chunks // 2
        cur, cur_chunks = [], 0
        split_done = False
        for op in rest_ops:
            if not split_done and cur_chunks >= half:
                blocks.append((cur, cur_chunks))
                cur, cur_chunks = [], 0
                split_done = True
            op = dict(op)
            op["q"] = cur_chunks
            cur.append(op)
            cur_chunks += op["w"]
        blocks.append((cur, cur_chunks))

    return blocks


def _build_bass_program(seg_chunks):
    """SPMD bass program computing per-class partial grams in fp8.

    Device input "z": class-sorted, zero-padded, PRE-TILED fp8 Z — for each
    DMA block of kb chunks a contiguous [128, kb*128] slab (each SBUF
    partition's data contiguous in DRAM). Output "m_out": [128, C*128]
    bf16 partial M (d on partitions, (j,e) on free).
    """
    import concourse.bacc as bacc
    import concourse.tile as tile
    from concourse import mybir
    from contextlib import ExitStack

    blocks = _plan(seg_chunks)
    total_chunks = sum(kb for _, kb in blocks)

    # Each PSUM bank is its OWN tile: classes 0-3 bank0, 4-7 bank1,
    # 8 bank2, 9 bank3. Draining a finished bank must not create a
    # write-after-read hazard on the banks the PE is still accumulating
    # into — with one big acc tile the Tile framework serializes every
    # subsequent matmul behind the drain copy (~1.4us stall per drain).
    # bank index, column offset within bank (in classes)
    psum_bank = {j: (0, j) for j in range(4)}
    psum_bank.update({j: (1, j - 4) for j in range(4, 8)})
    psum_bank[8] = (2, 0)
    psum_bank[9] = (3, 0)
    # store units: class range [a, b) stored when class b-1's cast is done.
    # Classes 8+9 ship as ONE ring-split store: flight time is descriptor-
    # count bound, so two half-stores per ring would cost double.
    stores = {3: (0, 4), 7: (4, 8), 9: (8, 10)}

    nc = bacc.Bacc("TRN2", target_bir_lowering=False, debug=False, num_devices=N_CORES)
    z = nc.dram_tensor(
        "z", [total_chunks * CHUNK, D], mybir.dt.float8e4, kind="ExternalInput"
    ).ap()
    out = nc.dram_tensor(
        "m_out", [D, C * D], mybir.dt.bfloat16, kind="ExternalOutput"
    ).ap()

    with tile.TileContext(nc) as tc:
        with ExitStack() as ctx:
            psum = ctx.enter_context(tc.tile_pool(name="psum", bufs=1, space="PSUM"))
            opool = ctx.enter_context(tc.tile_pool(name="o", bufs=1))

            banks = [
                psum.tile([128, 4 * D], mybir.dt.float32, name=f"bank{i}")
                for i in range(4)
            ]
            sb_out = opool.tile([128, C * D], mybir.dt.bfloat16)
            row0 = 0
            for b, (ops, kb) in enumerate(blocks):
                pool = ctx.enter_context(tc.tile_pool(name=f"z{b}", bufs=1))
                tl = pool.tile([128, kb * D], mybir.dt.float8e4)
                src = z[row0 : row0 + CHUNK * kb, :].rearrange(
                    "(p k) d -> p (k d)", p=128
                )
                nc.sync.dma_start(tl[0:64, :], src[0:64, :])
                nc.scalar.dma_start(tl[64:128, :], src[64:128, :])
                row0 += CHUNK * kb
                for op in ops:
                    j, q = op["cls"], op["q"]
                    bk, col = psum_bank[j]
                    dst = banks[bk][:, col * D : (col + 1) * D]
                    if op["w"] == 2:
                        opnd = tl[:, q * D : (q + 2) * D].rearrange(
                            "p (two d) -> p two d", two=2
                        )
                        nc.tensor.matmul(
                            dst,
                            opnd,
                            opnd,
                            start=op["start"],
                            stop=op["stop"],
                            perf_mode=mybir.MatmulPerfMode.DoubleRow,
                            skip_group_check=True,
                        )
                    else:
                        opnd = tl[:, q * D : (q + 1) * D]
                        nc.tensor.matmul(
                            dst,
                            opnd,
                            opnd,
                            start=op["start"],
                            stop=op["stop"],
                            skip_group_check=True,
                        )
                    # cast a PSUM bank out only when the whole bank is done
                    # — casting a single class early would create a write-
                    # after-read hazard with the PE still accumulating the
                    # bank's other classes; mid-stream stores ride SWDGE so
                    # they never steal load-ring bandwidth
                    if op["stop"]:
                        bk, col = psum_bank[j]
                        if j in (3, 7, 8, 9):
                            a0 = {3: 0, 7: 4, 8: 8, 9: 9}[j]
                            bk, col = psum_bank[a0]
                            csl = slice(a0 * D, (j + 1) * D)
                            nc.vector.tensor_copy(
                                sb_out[:, csl],
                                banks[bk][:, col * D : (col + (j + 1 - a0)) * D],
                            )
                        if j in stores:
                            a, bcls = stores[j]
                            sl = slice(a * D, bcls * D)
                            if j < C - 1:
                                nc.gpsimd.dma_start(out[:, sl], sb_out[:, sl])
                            else:
                                # the final store is the one on the critical
                                # path and its flight time is descriptor-
                                # count bound, so split its partition rows
                                # across both HW rings (input long done)
                                nc.sync.dma_start(
                                    out[0:64, sl], sb_out[0:64, sl]
                                )
                                nc.scalar.dma_start(
                                    out[64:128, sl], sb_out[64:128, sl]
                                )
    nc.compile()
    return nc


def _is_one_hot(Pi):
    if not (Pi.sum(axis=1) == 1.0).all():
        return False
    if not (Pi.max(axis=1) == 1.0).all():
        return False
    return np.count_nonzero(Pi) == Pi.shape[0]


def _fast_path_M(Z, Pi):
    """Per-class second moments via the device. Returns M [C, D, D] fp64."""
    from concourse.bass_utils import run_bass_kernel_spmd
    from concourse import mybir

    fp8 = mybir.dt.np(mybir.dt.float8e4)
    labels = np.argmax(Pi, axis=1)

    # balance every class across cores: class j's rows are dealt out in
    # near-equal contiguous slices, so per-class per-core counts differ by
    # at most 1 and padding is minimal
    order = np.argsort(labels, kind="stable")
    cls_counts = np.bincount(labels, minlength=C)
    cls_offs = np.concatenate([[0], np.cumsum(cls_counts)])

    counts = np.zeros((N_CORES, C), dtype=np.int64)
    for j in range(C):
        base, rem = divmod(int(cls_counts[j]), N_CORES)
        for c in range(N_CORES):
            counts[c, j] = base + (1 if c < rem else 0)

    seg_chunks = [max(1, int(np.ceil(counts[:, j].max() / CHUNK))) for j in range(C)]

    key = tuple(seg_chunks)
    if key not in _compiled_cache:
        _compiled_cache[key] = _build_bass_program(seg_chunks)
    nc = _compiled_cache[key]

    blocks = _plan(seg_chunks)
    block_sizes = [kb for _, kb in blocks]
    total_chunks = sum(block_sizes)
    offs = np.concatenate([[0], np.cumsum(seg_chunks)]) * CHUNK

    Zq = np.ascontiguousarray(Z, dtype=np.float32).astype(fp8)
    in_maps = []
    for c in range(N_CORES):
        zbuf = np.zeros((total_chunks * CHUNK, D), dtype=fp8)
        for j in range(C):
            lo = cls_offs[j] + counts[:c, j].sum()
            nj = counts[c, j]
            zbuf[offs[j] : offs[j] + nj] = Zq[order[lo : lo + nj]]
        # pre-tile each DMA block: [kb, 128, D] -> [128, kb*D]
        parts = []
        start = 0
        for kb in block_sizes:
            blk = zbuf[start * CHUNK : (start + kb) * CHUNK]
            parts.append(
                np.ascontiguousarray(
                    blk.reshape(kb, CHUNK, D).transpose(1, 0, 2)
                ).reshape(-1)
            )
            start += kb
        zdev = np.concatenate(parts).reshape(total_chunks * CHUNK, D)
        in_maps.append({"z": zdev})

    res = run_bass_kernel_spmd(nc, in_maps, list(range(N_CORES)))
    M = np.zeros((C, D, D), dtype=np.float64)
    for c in range(N_CORES):
        o = res.results[c]["m_out"].astype(np.float64)  # [D, C*D]
        M += o.reshape(D, C, D).transpose(1, 0, 2)
    return M


def _dense_path_M(Z, Pi):
    """General dense Pi: host BLAS contraction. Returns (M, gram) fp64."""
    Zf = np.ascontiguousarray(Z, dtype=np.float32)
    A = (Pi[:, :, None].astype(np.float32) * Zf[:, None, :]).reshape(Zf.shape[0], -1)
    M = (A.T @ Zf).reshape(C, D, D).astype(np.float64)
    gram = (Zf.T @ Zf).astype(np.float64)
    return M, gram


def kernel(Z, Pi, Us):
    Z = np.asarray(Z, dtype=np.float32)
    Pi = np.asarray(Pi, dtype=np.float32)
    Us = np.asarray(Us, dtype=np.float32)
    n, d = Z.shape

    if n == N_TOTAL and d == D and Pi.shape == (n, C) and _is_one_hot(Pi):
        M = _fast_path_M(Z, Pi)
        gram = M.sum(axis=0)
    else:
        M, gram = _dense_path_M(Z, Pi)

    nf = float(n)
    df = float(d)

    A = np.eye(d, dtype=np.float64) + (df / (nf * EPS)) * gram
    sign, logabsdet = np.linalg.slogdet(A)
    loss_R = 0.5 * logabsdet

    trPi = Pi.astype(np.float64).sum(axis=0)
    col_norms_sq = (Us.astype(np.float64) ** 2).sum(axis=1)  # [C, d]
    with np.errstate(divide="ignore"):
        per_class = np.log1p((df / (trPi[:, None] * EPS)) * col_norms_sq).sum(axis=1)
    loss_Rc = ((trPi / (2.0 * nf)) * per_class).sum()

    Us64 = Us.astype(np.float64)
    UUt = np.einsum("jdk,jek->jde", Us64, Us64)
    loss_reg = 0.5 * MU * ((M - UUt) ** 2).sum()

    loss_obj = loss_R - loss_Rc - loss_reg
    return (
        np.float32(-loss_obj),
        np.float32(loss_R),
        np.float32(loss_Rc),
        np.float32(loss_reg),
    )
